# revision 1
# baseline (speedup 1.0000x reference)
"""Trainium2 Bass kernel for a 4-layer GraphConv GNN + mean-pool classifier.

kernel(**inputs) takes the FULL unsharded inputs (as produced by
setup_inputs(): h, src, dst, graph_ids, W1..W4, b1..b4, Wc1, bc1, Wc2, bc2)
and returns the full [32, 10] float32 output.

Distribution (8 NeuronCores, SPMD):
  - dst nodes sharded across cores (12544 rows/core); each core owns all
    in-edges of its shard (host-side edge partitioning of the static graph).
  - The scaled feature table hs = x * deg_out^-1/2 is replicated in every
    core's HBM in fp16 and re-replicated after each layer with an AllGather
    collective (the halo exchange).
  - Per 128-dst-node block: a Q7 dma_gather fetches the message rows
    hs[src]; the vector engine builds one-hot scatter matrices
    S[edge, dstslot] = (iota == dstmod) and the tensor engine accumulates
    aggT += M_chunk.T @ S_chunk into PSUM (fp32). deg_in^-1/2 scaling, the
    layer weight matmul (fp32), bias+relu (scalar engine), a transpose
    back to node-major and the deg_out^-1/2 rescale produce the next table
    shard.
  - Readout: per-block graph-mask matmuls accumulate per-graph sums and
    counts in PSUM; one [32,136] AllReduce combines cores; the tiny
    classifier head runs replicated on every core.
"""
import dataclasses
import numpy as np

import concourse.bacc as bacc
import concourse.mybir as mybir
import concourse.tile as tile

F32 = mybir.dt.float32
F32R = mybir.dt.float32r
F16 = mybir.dt.float16
I32 = mybir.dt.int32
I16 = mybir.dt.int16

N_CORES = 8
N_NODES = 100000
N_GRAPHS = 32
D = 128
N_CLASSES = 10
GB = 4                      # blocks per gather group
SHARD = 12544               # ceil(100000 / (128*8)) * 128
NB = SHARD // 128           # 98 blocks per core
NTAB = SHARD * N_CORES      # 100352 replicated table rows
NQ = 4                      # int16-indexable sub-tables
QSIZE = NTAB // NQ          # 25088
NG = -(-NB // GB)           # 25 gather groups


def _preprocess(src, dst, graph_ids):
    src = np.asarray(src); dst = np.asarray(dst)
    gid = np.asarray(graph_ids)
    own = dst // SHARD
    blk = (dst % SHARD) >> 7
    slot = dst & 127
    q = src // QSIZE
    lidx = (src % QSIZE).astype(np.int64)

    key = ((own * NB + blk) * NQ + q).astype(np.int64)
    order = np.argsort(key, kind="stable")
    key_s = key[order]
    lidx_s = lidx[order]
    slot_s = slot[order]
    nbuckets = N_CORES * NB * NQ
    counts = np.bincount(key_s, minlength=nbuckets)
    J = int(max(1, -(-counts.max() // 128)))
    starts = np.concatenate([[0], np.cumsum(counts)])

    eidx = np.zeros((N_CORES, NG, NQ, 128, GB * J * 128 // 16), dtype=np.int16)
    edm = np.full((N_CORES, NB, 128, NQ * J), -1.0, dtype=np.float16)
    flat = np.zeros((N_CORES, NB, NQ, J * 128), dtype=np.int16)
    for c in range(N_CORES):
        for b in range(NB):
            for qq in range(NQ):
                k = (c * NB + b) * NQ + qq
                n = counts[k]
                s = starts[k]
                flat[c, b, qq, :n] = lidx_s[s:s + n]
                dm = edm[c, b]
                ii = np.arange(n)
                dm[ii % 128, qq * J + ii // 128] = slot_s[s:s + n]
    for c in range(N_CORES):
        for g in range(NG):
            for qq in range(NQ):
                parts = []
                for b_i in range(GB):
                    b = g * GB + b_i
                    parts.append(flat[c, b, qq] if b < NB
                                 else np.zeros(J * 128, dtype=np.int16))
                fl = np.concatenate(parts)
                w = fl.reshape(-1, 16).T
                eidx[c, g, qq] = np.tile(w, (8, 1))

    gmask = np.zeros((N_CORES, NB, 128, 32), dtype=np.float32)
    for c in range(N_CORES):
        lo, hi = c * SHARD, min((c + 1) * SHARD, N_NODES)
        if hi > lo:
            gmask[c].reshape(SHARD, 32)[np.arange(hi - lo), gid[lo:hi]] = 1.0

    deg_out_g = np.bincount(src, minlength=NTAB).astype(np.int32)
    deg_in_g = np.bincount(dst, minlength=NTAB).astype(np.int32)
    deg_out = np.zeros((N_CORES, 128, NB), dtype=np.int32)
    deg_in = np.zeros((N_CORES, 128, NB), dtype=np.int32)
    for c in range(N_CORES):
        deg_out[c] = deg_out_g[c * SHARD:(c + 1) * SHARD].reshape(NB, 128).T
        deg_in[c] = deg_in_g[c * SHARD:(c + 1) * SHARD].reshape(NB, 128).T
    return dict(eidx=eidx, edm=edm, gmask=gmask, deg_out=deg_out,
                deg_in=deg_in, J=J)


def _build_kernel(J):
    C = NQ * J
    NI = GB * J * 128
    NIW = NI // 16

    nc = bacc.Bacc("TRN2", num_devices=N_CORES, debug=False)

    h_in = nc.dram_tensor("h_shard", [SHARD, 128], F32, kind="ExternalInput")
    eidx_in = nc.dram_tensor("eidx", [NG, NQ, 128, NIW], I16, kind="ExternalInput")
    edm_in = nc.dram_tensor("edm", [NB, 128, C], F16, kind="ExternalInput")
    gmask_in = nc.dram_tensor("gmask", [NB, 128, 32], F32, kind="ExternalInput")
    degout_in = nc.dram_tensor("deg_out", [128, NB], I32, kind="ExternalInput")
    degin_in = nc.dram_tensor("deg_in", [128, NB], I32, kind="ExternalInput")
    iota_in = nc.dram_tensor("iota", [128, 128], F16, kind="ExternalInput")
    ident_in = nc.dram_tensor("ident", [128, 128], F32, kind="ExternalInput")
    ones_in = nc.dram_tensor("ones", [128, 128], F32, kind="ExternalInput")
    W_in = nc.dram_tensor("Ws", [4, 128, 128], F32, kind="ExternalInput")
    b_in = nc.dram_tensor("bs", [4, 128, 1], F32, kind="ExternalInput")
    Wc1_in = nc.dram_tensor("Wc1", [128, 128], F32, kind="ExternalInput")
    bc1_in = nc.dram_tensor("bc1b", [32, 128], F32, kind="ExternalInput")
    Wc2_in = nc.dram_tensor("Wc2p", [128, 16], F32, kind="ExternalInput")
    bc2_in = nc.dram_tensor("bc2b", [32, 16], F32, kind="ExternalInput")

    out_t = nc.dram_tensor("out", [32, 16], F32, kind="ExternalOutput")

    stage = [nc.dram_tensor(f"stage{l}", [SHARD, 128], F16, kind="Internal")
             for l in range(4)]
    tabs = [nc.dram_tensor(f"tab{l}", [NTAB, 128], F16, kind="Internal",
                           addr_space="Shared") for l in range(4)]
    ar_in = nc.dram_tensor("ar_in", [32, 136], F32, kind="Internal")
    ar_out = nc.dram_tensor("ar_out", [32, 136], F32, kind="Internal",
                            addr_space="Shared")
    rg = [list(range(N_CORES))]

    with tile.TileContext(nc) as tc:
        with (
            tc.tile_pool(name="const", bufs=1) as cpool,
            tc.tile_pool(name="mq", bufs=2 * NQ) as mqpool,
            tc.tile_pool(name="sp", bufs=2) as spool,
            tc.tile_pool(name="ip", bufs=3) as ipool,
            tc.tile_pool(name="small", bufs=3) as smpool,
            tc.tile_pool(name="ps_mm", bufs=5, space="PSUM") as ps_mm,
            tc.tile_pool(name="ps_rd", bufs=1, space="PSUM") as ps_rd,
        ):
            iota_t = cpool.tile([128, 128], F16, tag="iota")
            nc.sync.dma_start(iota_t[:], iota_in[:])
            ident_t = cpool.tile([128, 128], F32, tag="ident")
            nc.sync.dma_start(ident_t[:], ident_in[:])
            ones_t = cpool.tile([128, 128], F32, tag="ones")
            nc.sync.dma_start(ones_t[:], ones_in[:])
            W_t = cpool.tile([128, 4 * 128], F32, tag="W")
            nc.sync.dma_start(W_t[:].rearrange("p (l d) -> p l d", l=4),
                              W_in[:].rearrange("l p d -> p l d"))
            b_t = cpool.tile([128, 4], F32, tag="b")
            nc.sync.dma_start(b_t[:].rearrange("p (l o) -> p l o", l=4),
                              b_in[:].rearrange("l p o -> p l o"))
            Wc1_t = cpool.tile([128, 128], F32, tag="Wc1")
            nc.sync.dma_start(Wc1_t[:], Wc1_in[:])
            bc1_t = cpool.tile([32, 128], F32, tag="bc1")
            nc.sync.dma_start(bc1_t[:], bc1_in[:])
            Wc2_t = cpool.tile([128, 16], F32, tag="Wc2")
            nc.sync.dma_start(Wc2_t[:], Wc2_in[:])
            bc2_t = cpool.tile([32, 16], F32, tag="bc2")
            nc.sync.dma_start(bc2_t[:], bc2_in[:])

            def inv_sqrt_deg(deg_dram, tag):
                di = smpool.tile([128, NB], I32, tag=tag + "i")
                nc.sync.dma_start(di[:], deg_dram[:])
                df = cpool.tile([128, NB], F32, tag=tag + "f")
                nc.vector.tensor_copy(out=df[:], in_=di[:])
                nc.vector.tensor_scalar_max(df[:], df[:], 1.0)
                nc.scalar.sqrt(df[:], df[:])
                nc.vector.reciprocal(out=df[:], in_=df[:])
                return df
            invout_t = inv_sqrt_deg(degout_in, "dout")
            invin_t = inv_sqrt_deg(degin_in, "din")

            invin_b = cpool.tile([128, SHARD], F32, tag="invinb")
            for b in range(NB):
                tmp = smpool.tile([128, 128], F32, tag="ib_tmp")
                nc.vector.tensor_scalar_mul(tmp[:], ones_t[:],
                                            invin_t[:, b:b + 1])
                tps = ps_mm.tile([128, 128], F32, tag="mm")
                nc.tensor.transpose(tps[:], tmp[:], ident_t[:])
                nc.scalar.copy(invin_b[:, b * 128:(b + 1) * 128], tps[:])

            for b in range(NB):
                ht = smpool.tile([128, 128], F32, tag="h0")
                nc.sync.dma_start(ht[:], h_in[b * 128:(b + 1) * 128, :])
                hb = smpool.tile([128, 128], F16, tag="h0s")
                nc.vector.tensor_scalar_mul(hb[:], ht[:],
                                            invout_t[:, b:b + 1])
                nc.sync.dma_start(stage[0][b * 128:(b + 1) * 128, :], hb[:])
            nc.gpsimd.collective_compute(
                "AllGather", mybir.AluOpType.bypass, replica_groups=rg,
                ins=[stage[0][:].opt()], outs=[tabs[0][:].opt()])

            rd_ps = ps_rd.tile([32, 128], F32, tag="rd")
            cnt_ps = ps_rd.tile([32, 1], F32, tag="cnt")
            for l in range(4):
                tab_prev = tabs[l]
                for g in range(NG):
                    mqs = []
                    for qq in range(NQ):
                        it = ipool.tile([128, NIW], I16, tag="idx")
                        nc.sync.dma_start(it[:], eidx_in[g, qq])
                        mt = mqpool.tile([128, GB * J * 128], F16, tag="mq")
                        nc.gpsimd.dma_gather(
                            out_ap=mt[:].rearrange("p (c j) -> p c j",
                                                   c=GB * J),
                            in_ap=tab_prev[qq * QSIZE:(qq + 1) * QSIZE, :],
                            idxs_ap=it[:],
                            num_idxs=NI,
                            num_idxs_reg=NI,
                            elem_size=128,
                            single_packet=False,
                        )
                        mqs.append(mt)
                    for b_i in range(GB):
                        b = g * GB + b_i
                        if b >= NB:
                            continue
                        dmt = ipool.tile([128, C], F16, tag="dm")
                        nc.sync.dma_start(dmt[:], edm_in[b])
                        st = spool.tile([128, C * 128], F16, tag="s")
                        nc.vector.tensor_tensor(
                            out=st[:].rearrange("p (c j) -> p c j", c=C),
                            in0=dataclasses.replace(
                                iota_t[:], ap=[iota_t[:].ap[0], [0, C],
                                               list(iota_t[:].ap[1])]),
                            in1=dataclasses.replace(
                                dmt[:], ap=[dmt[:].ap[0], list(dmt[:].ap[1]),
                                            [0, 128]]),
                            op=mybir.AluOpType.is_equal)
                        pt = ps_mm.tile([128, 128], F32, tag="mm")
                        for c in range(C):
                            qq, j = c // J, c % J
                            col = (b_i * J + j) * 128
                            nc.tensor.matmul(
                                out=pt[:],
                                lhsT=mqs[qq][:, col:col + 128],
                                rhs=st[:, c * 128:(c + 1) * 128],
                                start=(c == 0), stop=(c == C - 1))
                        at = smpool.tile([128, 128], F32, tag="aggS")
                        nc.vector.tensor_tensor(
                            out=at[:], in0=pt[:],
                            in1=invin_b[:, b * 128:(b + 1) * 128],
                            op=mybir.AluOpType.mult)
                        xps = ps_mm.tile([128, 128], F32, tag="mm")
                        nc.tensor.matmul(
                            out=xps[:],
                            lhsT=W_t[:, l * 128:(l + 1) * 128],
                            rhs=at[:],
                            start=True, stop=True)
                        xs = smpool.tile([128, 128], F32, tag="xs")
                        nc.scalar.activation(
                            xs[:], xps[:], mybir.ActivationFunctionType.Relu,
                            bias=b_t[:, l:l + 1], scale=1.0)
                        tps = ps_mm.tile([128, 128], F32, tag="mm")
                        nc.tensor.transpose(tps[:], xs[:], ident_t[:])
                        if l < 3:
                            xb = smpool.tile([128, 128], F16, tag="xb")
                            nc.vector.tensor_scalar_mul(
                                xb[:], tps[:], invout_t[:, b:b + 1])
                            nc.sync.dma_start(
                                stage[l + 1][b * 128:(b + 1) * 128, :], xb[:])
                        else:
                            xf = smpool.tile([128, 128], F32, tag="x4")
                            nc.vector.tensor_copy(out=xf[:], in_=tps[:])
                            gm = ipool.tile([128, 32], F32, tag="gm")
                            nc.sync.dma_start(gm[:], gmask_in[b])
                            nc.tensor.matmul(
                                out=rd_ps[:], lhsT=gm[:],
                                rhs=xf[:],
                                start=(b == 0), stop=(b == NB - 1))
                            nc.tensor.matmul(
                                out=cnt_ps[:], lhsT=gm[:],
                                rhs=ones_t[:, :1],
                                start=(b == 0), stop=(b == NB - 1))
                if l < 3:
                    nc.gpsimd.collective_compute(
                        "AllGather", mybir.AluOpType.bypass, replica_groups=rg,
                        ins=[stage[l + 1][:].opt()],
                        outs=[tabs[l + 1][:].opt()])

            art = smpool.tile([32, 136], F32, tag="art")
            nc.vector.tensor_copy(out=art[:, :128], in_=rd_ps[:])
            nc.vector.tensor_copy(out=art[:, 128:129], in_=cnt_ps[:])
            nc.vector.memset(art[:, 129:136], 0.0)
            nc.sync.dma_start(ar_in[:], art[:])
            nc.gpsimd.collective_compute(
                "AllReduce", mybir.AluOpType.add, replica_groups=rg,
                ins=[ar_in[:].opt()], outs=[ar_out[:].opt()])
            art2 = smpool.tile([32, 136], F32, tag="art2")
            nc.sync.dma_start(art2[:], ar_out[:])
            cnt = smpool.tile([32, 1], F32, tag="cinv")
            nc.vector.tensor_scalar_max(cnt[:], art2[:, 128:129], 1.0)
            nc.vector.reciprocal(out=cnt[:], in_=cnt[:])
            hg = smpool.tile([32, 128], F32, tag="hg")
            nc.vector.tensor_scalar_mul(hg[:], art2[:, :128], cnt[:])
            hgt_ps = ps_mm.tile([128, 32], F32, tag="mm")
            nc.tensor.transpose(hgt_ps[:], hg[:], ident_t[:32, :32])
            hgt = smpool.tile([128, 32], F32, tag="hgts")
            nc.scalar.copy(hgt[:], hgt_ps[:])
            o1_ps = ps_mm.tile([32, 128], F32, tag="mm")
            nc.tensor.matmul(out=o1_ps[:], lhsT=hgt[:],
                             rhs=Wc1_t[:], start=True, stop=True)
            o1 = smpool.tile([32, 128], F32, tag="o1s")
            nc.vector.tensor_tensor(out=o1[:], in0=o1_ps[:], in1=bc1_t[:],
                                    op=mybir.AluOpType.add)
            nc.vector.tensor_scalar_max(o1[:], o1[:], 0.0)
            o1t_ps = ps_mm.tile([128, 32], F32, tag="mm")
            nc.tensor.transpose(o1t_ps[:], o1[:], ident_t[:32, :32])
            o1t = smpool.tile([128, 32], F32, tag="o1ts")
            nc.scalar.copy(o1t[:], o1t_ps[:])
            o2_ps = ps_mm.tile([32, 16], F32, tag="mm")
            nc.tensor.matmul(out=o2_ps[:], lhsT=o1t[:],
                             rhs=Wc2_t[:], start=True, stop=True)
            o2 = smpool.tile([32, 16], F32, tag="o2s")
            nc.vector.tensor_tensor(out=o2[:], in0=o2_ps[:], in1=bc2_t[:],
                                    op=mybir.AluOpType.add)
            nc.sync.dma_start(out_t[:], o2[:])

    nc.compile()
    return nc


def _make_in_maps(h, src, dst, graph_ids, Ws, bs, Wc1, bc1, Wc2, bc2, pre):
    hpad = np.zeros((NTAB, 128), dtype=np.float32)
    hpad[:N_NODES] = np.asarray(h, dtype=np.float32)
    iota = np.ascontiguousarray(
        np.broadcast_to(np.arange(128, dtype=np.float32), (128, 128))
    ).astype(np.float16)
    ident = np.eye(128, dtype=np.float32)
    ones = np.ones((128, 128), dtype=np.float32)
    Wstack = np.stack([np.asarray(w, np.float32) for w in Ws])
    bstack = np.stack([np.asarray(b, np.float32).reshape(128, 1) for b in bs])
    bc1b = np.broadcast_to(np.asarray(bc1, np.float32), (32, 128)).copy()
    Wc2p = np.zeros((128, 16), np.float32)
    Wc2p[:, :N_CLASSES] = np.asarray(Wc2, np.float32)
    bc2b = np.zeros((32, 16), np.float32)
    bc2b[:, :N_CLASSES] = np.broadcast_to(
        np.asarray(bc2, np.float32), (32, N_CLASSES))

    in_maps = []
    for c in range(N_CORES):
        in_maps.append({
            "h_shard": hpad[c * SHARD:(c + 1) * SHARD],
            "eidx": pre["eidx"][c],
            "edm": pre["edm"][c],
            "gmask": pre["gmask"][c],
            "deg_out": pre["deg_out"][c],
            "deg_in": pre["deg_in"][c],
            "iota": iota,
            "ident": ident,
            "ones": ones,
            "Ws": Wstack,
            "bs": bstack,
            "Wc1": np.asarray(Wc1, np.float32),
            "bc1b": bc1b,
            "Wc2p": Wc2p,
            "bc2b": bc2b,
        })
    return in_maps


_CACHE = {}


def kernel(h, src, dst, graph_ids, W1, b1, W2, b2, W3, b3, W4, b4,
           Wc1, bc1, Wc2, bc2):
    pre = _preprocess(src, dst, graph_ids)
    J = pre["J"]
    if J not in _CACHE:
        _CACHE[J] = _build_kernel(J)
    nc = _CACHE[J]
    in_maps = _make_in_maps(h, src, dst, graph_ids,
                            [W1, W2, W3, W4], [b1, b2, b3, b4],
                            Wc1, bc1, Wc2, bc2, pre)
    from concourse.bass_utils import run_bass_kernel_spmd
    r = run_bass_kernel_spmd(nc, in_maps, core_ids=list(range(N_CORES)))
    return np.ascontiguousarray(
        r.results[0]["out"][:, :N_CLASSES]).astype(np.float32)



# revision 2
# speedup vs baseline: 1.6247x; 1.6247x over previous
"""Trainium2 Bass kernel for a 4-layer GraphConv GNN + mean-pool classifier.

kernel(**inputs) takes the FULL unsharded inputs (as produced by
setup_inputs(): h, src, dst, graph_ids, W1..W4, b1..b4, Wc1, bc1, Wc2, bc2)
and returns the full [32, 10] float32 output.

Distribution (8 NeuronCores, SPMD):
  - dst nodes sharded across cores (12544 rows/core); each core owns all
    in-edges of its shard (host-side edge partitioning of the static graph).
  - The scaled feature table hs = x * deg_out^-1/2 is replicated in every
    core's HBM in fp16 and re-replicated after each layer with an AllGather
    collective (the halo exchange).
  - Per 128-dst-node block: dma_gathers (one per int16-indexable
    sub-table, spread across the 4 SWDGE queues so their transfers run
    concurrently) fetch the message rows hs[src]; the vector engine
    builds one-hot scatter matrices
    S[edge, dstslot] = (iota == dstmod) and the tensor engine accumulates
    aggT += M_chunk.T @ S_chunk into PSUM (fp32). deg_in^-1/2 scaling, the
    layer weight matmul (fp32), bias+relu (scalar engine), a transpose
    back to node-major and the deg_out^-1/2 rescale produce the next table
    shard.
  - Readout: per-block graph-mask matmuls accumulate per-graph sums and
    counts in PSUM; one [32,136] AllReduce combines cores; the tiny
    classifier head runs replicated on every core.
"""
import dataclasses
import numpy as np

import concourse.bacc as bacc
import concourse.mybir as mybir
import concourse.tile as tile

F32 = mybir.dt.float32
F32R = mybir.dt.float32r
F16 = mybir.dt.float16
I32 = mybir.dt.int32
I16 = mybir.dt.int16

N_CORES = 8
N_NODES = 100000
N_GRAPHS = 32
D = 128
N_CLASSES = 10
GB = 4                      # blocks per gather group
SHARD = 12544               # ceil(100000 / (128*8)) * 128
NB = SHARD // 128           # 98 blocks per core
NTAB = SHARD * N_CORES      # 100352 replicated table rows
NQ = 4                      # int16-indexable sub-tables
QSIZE = NTAB // NQ          # 25088
NG = -(-NB // GB)           # 25 gather groups


def _preprocess(src, dst, graph_ids):
    src = np.asarray(src); dst = np.asarray(dst)
    gid = np.asarray(graph_ids)
    own = dst // SHARD
    blk = (dst % SHARD) >> 7
    slot = dst & 127
    q = src // QSIZE
    lidx = (src % QSIZE).astype(np.int64)

    key = ((own * NB + blk) * NQ + q).astype(np.int64)
    order = np.argsort(key, kind="stable")
    key_s = key[order]
    lidx_s = lidx[order]
    slot_s = slot[order]
    nbuckets = N_CORES * NB * NQ
    counts = np.bincount(key_s, minlength=nbuckets)
    J = int(max(1, -(-counts.max() // 128)))
    starts = np.concatenate([[0], np.cumsum(counts)])

    eidx = np.zeros((N_CORES, NG, NQ, 128, GB * J * 128 // 16), dtype=np.int16)
    edm = np.full((N_CORES, NB, 128, NQ * J), -1.0, dtype=np.float16)
    flat = np.zeros((N_CORES, NB, NQ, J * 128), dtype=np.int16)
    for c in range(N_CORES):
        for b in range(NB):
            for qq in range(NQ):
                k = (c * NB + b) * NQ + qq
                n = counts[k]
                s = starts[k]
                flat[c, b, qq, :n] = lidx_s[s:s + n]
                dm = edm[c, b]
                ii = np.arange(n)
                dm[ii % 128, qq * J + ii // 128] = slot_s[s:s + n]
    for c in range(N_CORES):
        for g in range(NG):
            for qq in range(NQ):
                parts = []
                for b_i in range(GB):
                    b = g * GB + b_i
                    parts.append(flat[c, b, qq] if b < NB
                                 else np.zeros(J * 128, dtype=np.int16))
                fl = np.concatenate(parts)
                w = fl.reshape(-1, 16).T
                eidx[c, g, qq] = np.tile(w, (8, 1))

    gmask = np.zeros((N_CORES, NB, 128, 32), dtype=np.float32)
    for c in range(N_CORES):
        lo, hi = c * SHARD, min((c + 1) * SHARD, N_NODES)
        if hi > lo:
            gmask[c].reshape(SHARD, 32)[np.arange(hi - lo), gid[lo:hi]] = 1.0

    deg_out_g = np.bincount(src, minlength=NTAB).astype(np.int32)
    deg_in_g = np.bincount(dst, minlength=NTAB).astype(np.int32)
    deg_out = np.zeros((N_CORES, 128, NB), dtype=np.int32)
    deg_in = np.zeros((N_CORES, 128, NB), dtype=np.int32)
    for c in range(N_CORES):
        deg_out[c] = deg_out_g[c * SHARD:(c + 1) * SHARD].reshape(NB, 128).T
        deg_in[c] = deg_in_g[c * SHARD:(c + 1) * SHARD].reshape(NB, 128).T
    return dict(eidx=eidx, edm=edm, gmask=gmask, deg_out=deg_out,
                deg_in=deg_in, J=J)


def _build_kernel(J):
    C = NQ * J
    NI = GB * J * 128
    NIW = NI // 16

    nc = bacc.Bacc("TRN2", num_devices=N_CORES, debug=False,
                   num_swdge_queues=4)

    h_in = nc.dram_tensor("h_shard", [SHARD, 128], F32, kind="ExternalInput")
    eidx_in = nc.dram_tensor("eidx", [NG, NQ, 128, NIW], I16, kind="ExternalInput")
    edm_in = nc.dram_tensor("edm", [NB, 128, C], F16, kind="ExternalInput")
    gmask_in = nc.dram_tensor("gmask", [NB, 128, 32], F32, kind="ExternalInput")
    degout_in = nc.dram_tensor("deg_out", [128, NB], I32, kind="ExternalInput")
    degin_in = nc.dram_tensor("deg_in", [128, NB], I32, kind="ExternalInput")
    iota_in = nc.dram_tensor("iota", [128, 128], F16, kind="ExternalInput")
    ident_in = nc.dram_tensor("ident", [128, 128], F32, kind="ExternalInput")
    ones_in = nc.dram_tensor("ones", [128, 128], F32, kind="ExternalInput")
    W_in = nc.dram_tensor("Ws", [4, 128, 128], F32, kind="ExternalInput")
    b_in = nc.dram_tensor("bs", [4, 128, 1], F32, kind="ExternalInput")
    Wc1_in = nc.dram_tensor("Wc1", [128, 128], F32, kind="ExternalInput")
    bc1_in = nc.dram_tensor("bc1b", [32, 128], F32, kind="ExternalInput")
    Wc2_in = nc.dram_tensor("Wc2p", [128, 16], F32, kind="ExternalInput")
    bc2_in = nc.dram_tensor("bc2b", [32, 16], F32, kind="ExternalInput")

    out_t = nc.dram_tensor("out", [32, 16], F32, kind="ExternalOutput")

    stage = [nc.dram_tensor(f"stage{l}", [SHARD, 128], F16, kind="Internal")
             for l in range(4)]
    tabs = [nc.dram_tensor(f"tab{l}", [NTAB, 128], F16, kind="Internal",
                           addr_space="Shared") for l in range(4)]
    ar_in = nc.dram_tensor("ar_in", [32, 136], F32, kind="Internal")
    ar_out = nc.dram_tensor("ar_out", [32, 136], F32, kind="Internal",
                            addr_space="Shared")
    rg = [list(range(N_CORES))]

    with tile.TileContext(nc) as tc:
        with (
            tc.tile_pool(name="const", bufs=1) as cpool,
            tc.tile_pool(name="mq", bufs=2 * NQ) as mqpool,
            tc.tile_pool(name="sp", bufs=2) as spool,
            tc.tile_pool(name="ip", bufs=6) as ipool,
            tc.tile_pool(name="small", bufs=3) as smpool,
            tc.tile_pool(name="ps_mm", bufs=5, space="PSUM") as ps_mm,
            tc.tile_pool(name="ps_rd", bufs=1, space="PSUM") as ps_rd,
        ):
            iota_t = cpool.tile([128, 128], F16, tag="iota")
            nc.sync.dma_start(iota_t[:], iota_in[:])
            ident_t = cpool.tile([128, 128], F32, tag="ident")
            nc.sync.dma_start(ident_t[:], ident_in[:])
            ones_t = cpool.tile([128, 128], F32, tag="ones")
            nc.sync.dma_start(ones_t[:], ones_in[:])
            W_t = cpool.tile([128, 4 * 128], F32, tag="W")
            nc.sync.dma_start(W_t[:].rearrange("p (l d) -> p l d", l=4),
                              W_in[:].rearrange("l p d -> p l d"))
            b_t = cpool.tile([128, 4], F32, tag="b")
            nc.sync.dma_start(b_t[:].rearrange("p (l o) -> p l o", l=4),
                              b_in[:].rearrange("l p o -> p l o"))
            Wc1_t = cpool.tile([128, 128], F32, tag="Wc1")
            nc.sync.dma_start(Wc1_t[:], Wc1_in[:])
            bc1_t = cpool.tile([32, 128], F32, tag="bc1")
            nc.sync.dma_start(bc1_t[:], bc1_in[:])
            Wc2_t = cpool.tile([128, 16], F32, tag="Wc2")
            nc.sync.dma_start(Wc2_t[:], Wc2_in[:])
            bc2_t = cpool.tile([32, 16], F32, tag="bc2")
            nc.sync.dma_start(bc2_t[:], bc2_in[:])

            def inv_sqrt_deg(deg_dram, tag):
                di = smpool.tile([128, NB], I32, tag=tag + "i")
                nc.sync.dma_start(di[:], deg_dram[:])
                df = cpool.tile([128, NB], F32, tag=tag + "f")
                nc.vector.tensor_copy(out=df[:], in_=di[:])
                nc.vector.tensor_scalar_max(df[:], df[:], 1.0)
                nc.scalar.sqrt(df[:], df[:])
                nc.vector.reciprocal(out=df[:], in_=df[:])
                return df
            invout_t = inv_sqrt_deg(degout_in, "dout")
            invin_t = inv_sqrt_deg(degin_in, "din")

            invin_b = cpool.tile([128, SHARD], F32, tag="invinb")
            for b in range(NB):
                tmp = smpool.tile([128, 128], F32, tag="ib_tmp")
                nc.vector.tensor_scalar_mul(tmp[:], ones_t[:],
                                            invin_t[:, b:b + 1])
                tps = ps_mm.tile([128, 128], F32, tag="mm")
                nc.tensor.transpose(tps[:], tmp[:], ident_t[:])
                nc.scalar.copy(invin_b[:, b * 128:(b + 1) * 128], tps[:])

            for b in range(NB):
                ht = smpool.tile([128, 128], F32, tag="h0")
                nc.sync.dma_start(ht[:], h_in[b * 128:(b + 1) * 128, :])
                hb = smpool.tile([128, 128], F16, tag="h0s")
                nc.vector.tensor_scalar_mul(hb[:], ht[:],
                                            invout_t[:, b:b + 1])
                nc.sync.dma_start(stage[0][b * 128:(b + 1) * 128, :], hb[:])
            nc.gpsimd.collective_compute(
                "AllGather", mybir.AluOpType.bypass, replica_groups=rg,
                ins=[stage[0][:].opt()], outs=[tabs[0][:].opt()])

            rd_ps = ps_rd.tile([32, 128], F32, tag="rd")
            cnt_ps = ps_rd.tile([32, 1], F32, tag="cnt")
            for l in range(4):
                tab_prev = tabs[l]
                for g in range(NG):
                    mqs = []
                    for qq in range(NQ):
                        it = ipool.tile([128, NIW], I16, tag="idx")
                        nc.sync.dma_start(it[:], eidx_in[g, qq])
                        mt = mqpool.tile([128, GB * J * 128], F16, tag="mq")
                        nc.gpsimd.dma_gather(
                            out_ap=mt[:].rearrange("p (c j) -> p c j",
                                                   c=GB * J),
                            in_ap=tab_prev[qq * QSIZE:(qq + 1) * QSIZE, :],
                            idxs_ap=it[:],
                            num_idxs=NI,
                            num_idxs_reg=NI,
                            elem_size=128,
                            single_packet=False,
                            queue_num=qq,
                        )
                        mqs.append(mt)
                    for b_i in range(GB):
                        b = g * GB + b_i
                        if b >= NB:
                            continue
                        dmt = ipool.tile([128, C], F16, tag="dm")
                        nc.sync.dma_start(dmt[:], edm_in[b])
                        st = spool.tile([128, C * 128], F16, tag="s")
                        nc.vector.tensor_tensor(
                            out=st[:].rearrange("p (c j) -> p c j", c=C),
                            in0=dataclasses.replace(
                                iota_t[:], ap=[iota_t[:].ap[0], [0, C],
                                               list(iota_t[:].ap[1])]),
                            in1=dataclasses.replace(
                                dmt[:], ap=[dmt[:].ap[0], list(dmt[:].ap[1]),
                                            [0, 128]]),
                            op=mybir.AluOpType.is_equal)
                        pt = ps_mm.tile([128, 128], F32, tag="mm")
                        for c in range(C):
                            qq, j = c // J, c % J
                            col = (b_i * J + j) * 128
                            nc.tensor.matmul(
                                out=pt[:],
                                lhsT=mqs[qq][:, col:col + 128],
                                rhs=st[:, c * 128:(c + 1) * 128],
                                start=(c == 0), stop=(c == C - 1))
                        at = smpool.tile([128, 128], F32, tag="aggS")
                        nc.vector.tensor_tensor(
                            out=at[:], in0=pt[:],
                            in1=invin_b[:, b * 128:(b + 1) * 128],
                            op=mybir.AluOpType.mult)
                        xps = ps_mm.tile([128, 128], F32, tag="mm")
                        nc.tensor.matmul(
                            out=xps[:],
                            lhsT=W_t[:, l * 128:(l + 1) * 128],
                            rhs=at[:],
                            start=True, stop=True)
                        xs = smpool.tile([128, 128], F32, tag="xs")
                        nc.scalar.activation(
                            xs[:], xps[:], mybir.ActivationFunctionType.Relu,
                            bias=b_t[:, l:l + 1], scale=1.0)
                        tps = ps_mm.tile([128, 128], F32, tag="mm")
                        nc.tensor.transpose(tps[:], xs[:], ident_t[:])
                        if l < 3:
                            xb = smpool.tile([128, 128], F16, tag="xb")
                            nc.vector.tensor_scalar_mul(
                                xb[:], tps[:], invout_t[:, b:b + 1])
                            nc.sync.dma_start(
                                stage[l + 1][b * 128:(b + 1) * 128, :], xb[:])
                        else:
                            xf = smpool.tile([128, 128], F32, tag="x4")
                            nc.vector.tensor_copy(out=xf[:], in_=tps[:])
                            gm = ipool.tile([128, 32], F32, tag="gm")
                            nc.sync.dma_start(gm[:], gmask_in[b])
                            nc.tensor.matmul(
                                out=rd_ps[:], lhsT=gm[:],
                                rhs=xf[:],
                                start=(b == 0), stop=(b == NB - 1))
                            nc.tensor.matmul(
                                out=cnt_ps[:], lhsT=gm[:],
                                rhs=ones_t[:, :1],
                                start=(b == 0), stop=(b == NB - 1))
                if l < 3:
                    nc.gpsimd.collective_compute(
                        "AllGather", mybir.AluOpType.bypass, replica_groups=rg,
                        ins=[stage[l + 1][:].opt()],
                        outs=[tabs[l + 1][:].opt()])

            art = smpool.tile([32, 136], F32, tag="art")
            nc.vector.tensor_copy(out=art[:, :128], in_=rd_ps[:])
            nc.vector.tensor_copy(out=art[:, 128:129], in_=cnt_ps[:])
            nc.vector.memset(art[:, 129:136], 0.0)
            nc.sync.dma_start(ar_in[:], art[:])
            nc.gpsimd.collective_compute(
                "AllReduce", mybir.AluOpType.add, replica_groups=rg,
                ins=[ar_in[:].opt()], outs=[ar_out[:].opt()])
            art2 = smpool.tile([32, 136], F32, tag="art2")
            nc.sync.dma_start(art2[:], ar_out[:])
            cnt = smpool.tile([32, 1], F32, tag="cinv")
            nc.vector.tensor_scalar_max(cnt[:], art2[:, 128:129], 1.0)
            nc.vector.reciprocal(out=cnt[:], in_=cnt[:])
            hg = smpool.tile([32, 128], F32, tag="hg")
            nc.vector.tensor_scalar_mul(hg[:], art2[:, :128], cnt[:])
            hgt_ps = ps_mm.tile([128, 32], F32, tag="mm")
            nc.tensor.transpose(hgt_ps[:], hg[:], ident_t[:32, :32])
            hgt = smpool.tile([128, 32], F32, tag="hgts")
            nc.scalar.copy(hgt[:], hgt_ps[:])
            o1_ps = ps_mm.tile([32, 128], F32, tag="mm")
            nc.tensor.matmul(out=o1_ps[:], lhsT=hgt[:],
                             rhs=Wc1_t[:], start=True, stop=True)
            o1 = smpool.tile([32, 128], F32, tag="o1s")
            nc.vector.tensor_tensor(out=o1[:], in0=o1_ps[:], in1=bc1_t[:],
                                    op=mybir.AluOpType.add)
            nc.vector.tensor_scalar_max(o1[:], o1[:], 0.0)
            o1t_ps = ps_mm.tile([128, 32], F32, tag="mm")
            nc.tensor.transpose(o1t_ps[:], o1[:], ident_t[:32, :32])
            o1t = smpool.tile([128, 32], F32, tag="o1ts")
            nc.scalar.copy(o1t[:], o1t_ps[:])
            o2_ps = ps_mm.tile([32, 16], F32, tag="mm")
            nc.tensor.matmul(out=o2_ps[:], lhsT=o1t[:],
                             rhs=Wc2_t[:], start=True, stop=True)
            o2 = smpool.tile([32, 16], F32, tag="o2s")
            nc.vector.tensor_tensor(out=o2[:], in0=o2_ps[:], in1=bc2_t[:],
                                    op=mybir.AluOpType.add)
            nc.sync.dma_start(out_t[:], o2[:])

    nc.compile()
    return nc


def _make_in_maps(h, src, dst, graph_ids, Ws, bs, Wc1, bc1, Wc2, bc2, pre):
    hpad = np.zeros((NTAB, 128), dtype=np.float32)
    hpad[:N_NODES] = np.asarray(h, dtype=np.float32)
    iota = np.ascontiguousarray(
        np.broadcast_to(np.arange(128, dtype=np.float32), (128, 128))
    ).astype(np.float16)
    ident = np.eye(128, dtype=np.float32)
    ones = np.ones((128, 128), dtype=np.float32)
    Wstack = np.stack([np.asarray(w, np.float32) for w in Ws])
    bstack = np.stack([np.asarray(b, np.float32).reshape(128, 1) for b in bs])
    bc1b = np.broadcast_to(np.asarray(bc1, np.float32), (32, 128)).copy()
    Wc2p = np.zeros((128, 16), np.float32)
    Wc2p[:, :N_CLASSES] = np.asarray(Wc2, np.float32)
    bc2b = np.zeros((32, 16), np.float32)
    bc2b[:, :N_CLASSES] = np.broadcast_to(
        np.asarray(bc2, np.float32), (32, N_CLASSES))

    in_maps = []
    for c in range(N_CORES):
        in_maps.append({
            "h_shard": hpad[c * SHARD:(c + 1) * SHARD],
            "eidx": pre["eidx"][c],
            "edm": pre["edm"][c],
            "gmask": pre["gmask"][c],
            "deg_out": pre["deg_out"][c],
            "deg_in": pre["deg_in"][c],
            "iota": iota,
            "ident": ident,
            "ones": ones,
            "Ws": Wstack,
            "bs": bstack,
            "Wc1": np.asarray(Wc1, np.float32),
            "bc1b": bc1b,
            "Wc2p": Wc2p,
            "bc2b": bc2b,
        })
    return in_maps


_CACHE = {}


def kernel(h, src, dst, graph_ids, W1, b1, W2, b2, W3, b3, W4, b4,
           Wc1, bc1, Wc2, bc2):
    pre = _preprocess(src, dst, graph_ids)
    J = pre["J"]
    if J not in _CACHE:
        _CACHE[J] = _build_kernel(J)
    nc = _CACHE[J]
    in_maps = _make_in_maps(h, src, dst, graph_ids,
                            [W1, W2, W3, W4], [b1, b2, b3, b4],
                            Wc1, bc1, Wc2, bc2, pre)
    from concourse.bass_utils import run_bass_kernel_spmd
    r = run_bass_kernel_spmd(nc, in_maps, core_ids=list(range(N_CORES)))
    return np.ascontiguousarray(
        r.results[0]["out"][:, :N_CLASSES]).astype(np.float32)



# revision 3
# speedup vs baseline: 1.9398x; 1.1939x over previous
"""Trainium2 Bass kernel for a 4-layer GraphConv GNN + mean-pool classifier.

kernel(**inputs) takes the FULL unsharded inputs (as produced by
setup_inputs(): h, src, dst, graph_ids, W1..W4, b1..b4, Wc1, bc1, Wc2, bc2)
and returns the full [32, 10] float32 output.

Distribution (8 NeuronCores, SPMD):
  - dst nodes sharded across cores (12544 rows/core); each core owns all
    in-edges of its shard (host-side edge partitioning of the static graph).
  - The scaled feature table hs = x * deg_out^-1/2 is replicated in every
    core's HBM in fp16 and re-replicated after each layer with an AllGather
    collective (the halo exchange).
  - Per 128-dst-node block: dma_gathers (one per int16-indexable
    sub-table, spread across the 4 SWDGE queues so their transfers run
    concurrently) fetch the message rows hs[src]; the vector engine
    builds one-hot scatter matrices
    S[edge, dstslot] = (iota == dstmod) and the tensor engine accumulates
    aggT += M_chunk.T @ S_chunk into PSUM (fp32). deg_in^-1/2 scaling, the
    layer weight matmul (fp32), bias+relu (scalar engine), a transpose
    back to node-major and the deg_out^-1/2 rescale produce the next table
    shard.
  - Readout: per-block graph-mask matmuls accumulate per-graph sums and
    counts in PSUM; one [32,136] AllReduce combines cores; the tiny
    classifier head runs replicated on every core.
"""
import dataclasses
import numpy as np

import concourse.bacc as bacc
import concourse.mybir as mybir
import concourse.tile as tile

F32 = mybir.dt.float32
F32R = mybir.dt.float32r
F16 = mybir.dt.float16
I32 = mybir.dt.int32
I16 = mybir.dt.int16

N_CORES = 8
N_NODES = 100000
N_GRAPHS = 32
D = 128
N_CLASSES = 10
GB = 4                      # blocks per gather group
SHARD = 12544               # ceil(100000 / (128*8)) * 128
NB = SHARD // 128           # 98 blocks per core
NTAB = SHARD * N_CORES      # 100352 replicated table rows
NQ = 4                      # int16-indexable sub-tables
QSIZE = NTAB // NQ          # 25088
NG = -(-NB // GB)           # 25 gather groups


def _preprocess(src, dst, graph_ids):
    src = np.asarray(src); dst = np.asarray(dst)
    gid = np.asarray(graph_ids)
    own = dst // SHARD
    blk = (dst % SHARD) >> 7
    slot = dst & 127
    q = src // QSIZE
    lidx = (src % QSIZE).astype(np.int64)

    key = ((own * NB + blk) * NQ + q).astype(np.int64)
    order = np.argsort(key, kind="stable")
    key_s = key[order]
    lidx_s = lidx[order]
    slot_s = slot[order]
    nbuckets = N_CORES * NB * NQ
    counts = np.bincount(key_s, minlength=nbuckets)
    J = int(max(1, -(-counts.max() // 128)))
    starts = np.concatenate([[0], np.cumsum(counts)])

    eidx = np.zeros((N_CORES, NG, NQ, 128, GB * J * 128 // 16), dtype=np.int16)
    edm = np.full((N_CORES, NB, 128, NQ * J), -1.0, dtype=np.float16)
    flat = np.zeros((N_CORES, NB, NQ, J * 128), dtype=np.int16)
    for c in range(N_CORES):
        for b in range(NB):
            for qq in range(NQ):
                k = (c * NB + b) * NQ + qq
                n = counts[k]
                s = starts[k]
                flat[c, b, qq, :n] = lidx_s[s:s + n]
                dm = edm[c, b]
                ii = np.arange(n)
                dm[ii % 128, qq * J + ii // 128] = slot_s[s:s + n]
    for c in range(N_CORES):
        for g in range(NG):
            for qq in range(NQ):
                parts = []
                for b_i in range(GB):
                    b = g * GB + b_i
                    parts.append(flat[c, b, qq] if b < NB
                                 else np.zeros(J * 128, dtype=np.int16))
                fl = np.concatenate(parts)
                w = fl.reshape(-1, 16).T
                eidx[c, g, qq] = np.tile(w, (8, 1))

    gmask = np.zeros((N_CORES, NB, 128, 32), dtype=np.float32)
    for c in range(N_CORES):
        lo, hi = c * SHARD, min((c + 1) * SHARD, N_NODES)
        if hi > lo:
            gmask[c].reshape(SHARD, 32)[np.arange(hi - lo), gid[lo:hi]] = 1.0

    deg_out_g = np.bincount(src, minlength=NTAB).astype(np.int32)
    deg_in_g = np.bincount(dst, minlength=NTAB).astype(np.int32)
    deg_out = np.zeros((N_CORES, 128, NB), dtype=np.int32)
    deg_in = np.zeros((N_CORES, 128, NB), dtype=np.int32)
    for c in range(N_CORES):
        deg_out[c] = deg_out_g[c * SHARD:(c + 1) * SHARD].reshape(NB, 128).T
        deg_in[c] = deg_in_g[c * SHARD:(c + 1) * SHARD].reshape(NB, 128).T
    return dict(eidx=eidx, edm=edm, gmask=gmask, deg_out=deg_out,
                deg_in=deg_in, J=J)


def _build_kernel(J):
    C = NQ * J
    NI = GB * J * 128
    NIW = NI // 16

    nc = bacc.Bacc("TRN2", num_devices=N_CORES, debug=False,
                   num_swdge_queues=4)

    h_in = nc.dram_tensor("h_shard", [SHARD, 128], F32, kind="ExternalInput")
    eidx_in = nc.dram_tensor("eidx", [NG, NQ, 128, NIW], I16, kind="ExternalInput")
    edm_in = nc.dram_tensor("edm", [NB, 128, C], F16, kind="ExternalInput")
    gmask_in = nc.dram_tensor("gmask", [NB, 128, 32], F32, kind="ExternalInput")
    degout_in = nc.dram_tensor("deg_out", [128, NB], I32, kind="ExternalInput")
    degin_in = nc.dram_tensor("deg_in", [128, NB], I32, kind="ExternalInput")
    iota_in = nc.dram_tensor("iota", [128, 128], F16, kind="ExternalInput")
    ident_in = nc.dram_tensor("ident", [128, 128], F32, kind="ExternalInput")
    ones_in = nc.dram_tensor("ones", [128, 128], F32, kind="ExternalInput")
    W_in = nc.dram_tensor("Ws", [4, 128, 128], F32, kind="ExternalInput")
    b_in = nc.dram_tensor("bs", [4, 128, 1], F32, kind="ExternalInput")
    Wc1_in = nc.dram_tensor("Wc1", [128, 128], F32, kind="ExternalInput")
    bc1_in = nc.dram_tensor("bc1b", [32, 128], F32, kind="ExternalInput")
    Wc2_in = nc.dram_tensor("Wc2p", [128, 16], F32, kind="ExternalInput")
    bc2_in = nc.dram_tensor("bc2b", [32, 16], F32, kind="ExternalInput")

    out_t = nc.dram_tensor("out", [32, 16], F32, kind="ExternalOutput")

    stage = [nc.dram_tensor(f"stage{l}", [SHARD, 128], F16, kind="Internal")
             for l in range(4)]
    tabs = [nc.dram_tensor(f"tab{l}", [NTAB, 128], F16, kind="Internal",
                           addr_space="Shared") for l in range(4)]
    ar_in = nc.dram_tensor("ar_in", [32, 136], F32, kind="Internal")
    ar_out = nc.dram_tensor("ar_out", [32, 136], F32, kind="Internal",
                            addr_space="Shared")
    rg = [list(range(N_CORES))]

    with tile.TileContext(nc) as tc:
        with (
            tc.tile_pool(name="const", bufs=1) as cpool,
            tc.tile_pool(name="mq", bufs=10) as mqpool,
            tc.tile_pool(name="sp", bufs=2) as spool,
            tc.tile_pool(name="ip", bufs=6) as ipool,
            tc.tile_pool(name="small", bufs=3) as smpool,
            tc.tile_pool(name="ps_mm", bufs=5, space="PSUM") as ps_mm,
            tc.tile_pool(name="ps_rd", bufs=1, space="PSUM") as ps_rd,
        ):
            iota_t = cpool.tile([128, 128], F16, tag="iota")
            nc.sync.dma_start(iota_t[:], iota_in[:])
            ident_t = cpool.tile([128, 128], F32, tag="ident")
            nc.sync.dma_start(ident_t[:], ident_in[:])
            ones_t = cpool.tile([128, 128], F32, tag="ones")
            nc.sync.dma_start(ones_t[:], ones_in[:])
            W_t = cpool.tile([128, 4 * 128], F32, tag="W")
            nc.sync.dma_start(W_t[:].rearrange("p (l d) -> p l d", l=4),
                              W_in[:].rearrange("l p d -> p l d"))
            b_t = cpool.tile([128, 4], F32, tag="b")
            nc.sync.dma_start(b_t[:].rearrange("p (l o) -> p l o", l=4),
                              b_in[:].rearrange("l p o -> p l o"))
            Wc1_t = cpool.tile([128, 128], F32, tag="Wc1")
            nc.sync.dma_start(Wc1_t[:], Wc1_in[:])
            bc1_t = cpool.tile([32, 128], F32, tag="bc1")
            nc.sync.dma_start(bc1_t[:], bc1_in[:])
            Wc2_t = cpool.tile([128, 16], F32, tag="Wc2")
            nc.sync.dma_start(Wc2_t[:], Wc2_in[:])
            bc2_t = cpool.tile([32, 16], F32, tag="bc2")
            nc.sync.dma_start(bc2_t[:], bc2_in[:])

            def inv_sqrt_deg(deg_dram, tag):
                di = smpool.tile([128, NB], I32, tag=tag + "i")
                nc.sync.dma_start(di[:], deg_dram[:])
                df = cpool.tile([128, NB], F32, tag=tag + "f")
                nc.vector.tensor_copy(out=df[:], in_=di[:])
                nc.vector.tensor_scalar_max(df[:], df[:], 1.0)
                nc.scalar.sqrt(df[:], df[:])
                nc.vector.reciprocal(out=df[:], in_=df[:])
                return df
            invout_t = inv_sqrt_deg(degout_in, "dout")
            invin_t = inv_sqrt_deg(degin_in, "din")

            invin_b = cpool.tile([128, SHARD], F16, tag="invinb")
            for b in range(NB):
                tmp = smpool.tile([128, 128], F32, tag="ib_tmp")
                nc.vector.tensor_scalar_mul(tmp[:], ones_t[:],
                                            invin_t[:, b:b + 1])
                tps = ps_mm.tile([128, 128], F32, tag="mm")
                nc.tensor.transpose(tps[:], tmp[:], ident_t[:])
                nc.scalar.copy(invin_b[:, b * 128:(b + 1) * 128], tps[:])

            for b in range(NB):
                ht = smpool.tile([128, 128], F32, tag="h0")
                nc.sync.dma_start(ht[:], h_in[b * 128:(b + 1) * 128, :])
                hb = smpool.tile([128, 128], F16, tag="h0s")
                nc.vector.tensor_scalar_mul(hb[:], ht[:],
                                            invout_t[:, b:b + 1])
                nc.sync.dma_start(stage[0][b * 128:(b + 1) * 128, :], hb[:])
            nc.gpsimd.collective_compute(
                "AllGather", mybir.AluOpType.bypass, replica_groups=rg,
                ins=[stage[0][:].opt()], outs=[tabs[0][:].opt()])

            rd_ps = ps_rd.tile([32, 128], F32, tag="rd")
            cnt_ps = ps_rd.tile([32, 1], F32, tag="cnt")
            for l in range(4):
                tab_prev = tabs[l]
                for g in range(NG):
                    mqs = []
                    for qq in range(NQ):
                        it = ipool.tile([128, NIW], I16, tag="idx")
                        nc.sync.dma_start(it[:], eidx_in[g, qq])
                        mt = mqpool.tile([128, GB * J * 128], F16, tag="mq")
                        nc.gpsimd.dma_gather(
                            out_ap=mt[:].rearrange("p (c j) -> p c j",
                                                   c=GB * J),
                            in_ap=tab_prev[qq * QSIZE:(qq + 1) * QSIZE, :],
                            idxs_ap=it[:],
                            num_idxs=NI,
                            num_idxs_reg=NI,
                            elem_size=128,
                            single_packet=False,
                            queue_num=qq,
                        )
                        mqs.append(mt)
                    for b_i in range(GB):
                        b = g * GB + b_i
                        if b >= NB:
                            continue
                        dmt = ipool.tile([128, C], F16, tag="dm")
                        nc.sync.dma_start(dmt[:], edm_in[b])
                        st = spool.tile([128, C * 128], F16, tag="s")
                        nc.vector.tensor_tensor(
                            out=st[:].rearrange("p (c j) -> p c j", c=C),
                            in0=dataclasses.replace(
                                iota_t[:], ap=[iota_t[:].ap[0], [0, C],
                                               list(iota_t[:].ap[1])]),
                            in1=dataclasses.replace(
                                dmt[:], ap=[dmt[:].ap[0], list(dmt[:].ap[1]),
                                            [0, 128]]),
                            op=mybir.AluOpType.is_equal)
                        pt = ps_mm.tile([128, 128], F32, tag="mm")
                        for c in range(C):
                            qq, j = c // J, c % J
                            col = (b_i * J + j) * 128
                            nc.tensor.matmul(
                                out=pt[:],
                                lhsT=mqs[qq][:, col:col + 128],
                                rhs=st[:, c * 128:(c + 1) * 128],
                                start=(c == 0), stop=(c == C - 1))
                        at = smpool.tile([128, 128], F32, tag="aggS")
                        nc.vector.tensor_tensor(
                            out=at[:], in0=pt[:],
                            in1=invin_b[:, b * 128:(b + 1) * 128],
                            op=mybir.AluOpType.mult)
                        xps = ps_mm.tile([128, 128], F32, tag="mm")
                        nc.tensor.matmul(
                            out=xps[:],
                            lhsT=W_t[:, l * 128:(l + 1) * 128],
                            rhs=at[:],
                            start=True, stop=True)
                        xs = smpool.tile([128, 128], F32, tag="xs")
                        nc.scalar.activation(
                            xs[:], xps[:], mybir.ActivationFunctionType.Relu,
                            bias=b_t[:, l:l + 1], scale=1.0)
                        tps = ps_mm.tile([128, 128], F32, tag="mm")
                        nc.tensor.transpose(tps[:], xs[:], ident_t[:])
                        if l < 3:
                            xb = smpool.tile([128, 128], F16, tag="xb")
                            nc.vector.tensor_scalar_mul(
                                xb[:], tps[:], invout_t[:, b:b + 1])
                            nc.sync.dma_start(
                                stage[l + 1][b * 128:(b + 1) * 128, :], xb[:])
                        else:
                            xf = smpool.tile([128, 128], F32, tag="x4")
                            nc.vector.tensor_copy(out=xf[:], in_=tps[:])
                            gm = ipool.tile([128, 32], F32, tag="gm")
                            nc.sync.dma_start(gm[:], gmask_in[b])
                            nc.tensor.matmul(
                                out=rd_ps[:], lhsT=gm[:],
                                rhs=xf[:],
                                start=(b == 0), stop=(b == NB - 1))
                            nc.tensor.matmul(
                                out=cnt_ps[:], lhsT=gm[:],
                                rhs=ones_t[:, :1],
                                start=(b == 0), stop=(b == NB - 1))
                if l < 3:
                    nc.gpsimd.collective_compute(
                        "AllGather", mybir.AluOpType.bypass, replica_groups=rg,
                        ins=[stage[l + 1][:].opt()],
                        outs=[tabs[l + 1][:].opt()])

            art = smpool.tile([32, 136], F32, tag="art")
            nc.vector.tensor_copy(out=art[:, :128], in_=rd_ps[:])
            nc.vector.tensor_copy(out=art[:, 128:129], in_=cnt_ps[:])
            nc.vector.memset(art[:, 129:136], 0.0)
            nc.sync.dma_start(ar_in[:], art[:])
            nc.gpsimd.collective_compute(
                "AllReduce", mybir.AluOpType.add, replica_groups=rg,
                ins=[ar_in[:].opt()], outs=[ar_out[:].opt()])
            art2 = smpool.tile([32, 136], F32, tag="art2")
            nc.sync.dma_start(art2[:], ar_out[:])
            cnt = smpool.tile([32, 1], F32, tag="cinv")
            nc.vector.tensor_scalar_max(cnt[:], art2[:, 128:129], 1.0)
            nc.vector.reciprocal(out=cnt[:], in_=cnt[:])
            hg = smpool.tile([32, 128], F32, tag="hg")
            nc.vector.tensor_scalar_mul(hg[:], art2[:, :128], cnt[:])
            hgt_ps = ps_mm.tile([128, 32], F32, tag="mm")
            nc.tensor.transpose(hgt_ps[:], hg[:], ident_t[:32, :32])
            hgt = smpool.tile([128, 32], F32, tag="hgts")
            nc.scalar.copy(hgt[:], hgt_ps[:])
            o1_ps = ps_mm.tile([32, 128], F32, tag="mm")
            nc.tensor.matmul(out=o1_ps[:], lhsT=hgt[:],
                             rhs=Wc1_t[:], start=True, stop=True)
            o1 = smpool.tile([32, 128], F32, tag="o1s")
            nc.vector.tensor_tensor(out=o1[:], in0=o1_ps[:], in1=bc1_t[:],
                                    op=mybir.AluOpType.add)
            nc.vector.tensor_scalar_max(o1[:], o1[:], 0.0)
            o1t_ps = ps_mm.tile([128, 32], F32, tag="mm")
            nc.tensor.transpose(o1t_ps[:], o1[:], ident_t[:32, :32])
            o1t = smpool.tile([128, 32], F32, tag="o1ts")
            nc.scalar.copy(o1t[:], o1t_ps[:])
            o2_ps = ps_mm.tile([32, 16], F32, tag="mm")
            nc.tensor.matmul(out=o2_ps[:], lhsT=o1t[:],
                             rhs=Wc2_t[:], start=True, stop=True)
            o2 = smpool.tile([32, 16], F32, tag="o2s")
            nc.vector.tensor_tensor(out=o2[:], in0=o2_ps[:], in1=bc2_t[:],
                                    op=mybir.AluOpType.add)
            nc.sync.dma_start(out_t[:], o2[:])

    nc.compile()
    return nc


def _make_in_maps(h, src, dst, graph_ids, Ws, bs, Wc1, bc1, Wc2, bc2, pre):
    hpad = np.zeros((NTAB, 128), dtype=np.float32)
    hpad[:N_NODES] = np.asarray(h, dtype=np.float32)
    iota = np.ascontiguousarray(
        np.broadcast_to(np.arange(128, dtype=np.float32), (128, 128))
    ).astype(np.float16)
    ident = np.eye(128, dtype=np.float32)
    ones = np.ones((128, 128), dtype=np.float32)
    Wstack = np.stack([np.asarray(w, np.float32) for w in Ws])
    bstack = np.stack([np.asarray(b, np.float32).reshape(128, 1) for b in bs])
    bc1b = np.broadcast_to(np.asarray(bc1, np.float32), (32, 128)).copy()
    Wc2p = np.zeros((128, 16), np.float32)
    Wc2p[:, :N_CLASSES] = np.asarray(Wc2, np.float32)
    bc2b = np.zeros((32, 16), np.float32)
    bc2b[:, :N_CLASSES] = np.broadcast_to(
        np.asarray(bc2, np.float32), (32, N_CLASSES))

    in_maps = []
    for c in range(N_CORES):
        in_maps.append({
            "h_shard": hpad[c * SHARD:(c + 1) * SHARD],
            "eidx": pre["eidx"][c],
            "edm": pre["edm"][c],
            "gmask": pre["gmask"][c],
            "deg_out": pre["deg_out"][c],
            "deg_in": pre["deg_in"][c],
            "iota": iota,
            "ident": ident,
            "ones": ones,
            "Ws": Wstack,
            "bs": bstack,
            "Wc1": np.asarray(Wc1, np.float32),
            "bc1b": bc1b,
            "Wc2p": Wc2p,
            "bc2b": bc2b,
        })
    return in_maps


_CACHE = {}


def kernel(h, src, dst, graph_ids, W1, b1, W2, b2, W3, b3, W4, b4,
           Wc1, bc1, Wc2, bc2):
    pre = _preprocess(src, dst, graph_ids)
    J = pre["J"]
    if J not in _CACHE:
        _CACHE[J] = _build_kernel(J)
    nc = _CACHE[J]
    in_maps = _make_in_maps(h, src, dst, graph_ids,
                            [W1, W2, W3, W4], [b1, b2, b3, b4],
                            Wc1, bc1, Wc2, bc2, pre)
    from concourse.bass_utils import run_bass_kernel_spmd
    r = run_bass_kernel_spmd(nc, in_maps, core_ids=list(range(N_CORES)))
    return np.ascontiguousarray(
        r.results[0]["out"][:, :N_CLASSES]).astype(np.float32)

